# revision 5
# baseline (speedup 1.0000x reference)
"""LISTA / AtasiNet Trainium2 kernel.

Math (reference): K=10 iterations of
    Z     = gamma - D @ (beta_k * W)          # [B,N]
    theta = mu_k / (|Z| + EPS)
    gamma = sign(Z) * max(|Z| - theta, 0)
    D     = gamma @ A.T - y                   # [B,M]

Distribution: pure data-parallel over batch B=2048 across 8 NeuronCores
(B_local=256 per core); A, W, mu, beta replicated. No collectives.

Per-core layout is fully transposed (batch on the free axis):
    gammaT [N=4096, B=256]   DT [M=1024, B=256]
so both matmuls take naturally-laid-out weights:
    Z^T = gammaT + (-W)^T @ (beta_k D)^T : lhsT = -W tile  [m128, n128]
    D^T = A @ gammaT                     : lhsT = A.T tile [n128, m128]
A.T and -W are precomputed on host; matmul inputs are bf16 with f32 PSUM
accumulation. The gamma subtraction runs on VectorE (which has slack) for
iterations 2-8; in the final iteration - where VectorE would be the pacer
because there is no D update - it is instead folded into the matmul PSUM
accumulation group as an identity matmul of the bf16 gamma state.

Iteration k=0 is folded out analytically: for any mu_0 >= 0 the first
iteration yields gamma=0, D=-y. Device loop runs k=1..9; the last
iteration skips the (unused) D update and writes gamma out in f32.

Elementwise threshold uses the multiplicative form
    gamma = Z * relu(1 - mu_k / (|Z| (|Z|+EPS)))
as three VectorE ops per tile: a custom fused DVE op for
p = max(|Z|(|Z|+EPS), 1e-30), reciprocal_approx_fast (~51 ULP), and a
custom fused relu-affine-multiply writing the bf16 gamma state. The final
iteration (no D update, so VectorE would pace it) moves the p computation
to ScalarE via Square(|Z| + EPS/2) - EPS^2/4.

All DRAM inputs/outputs use partition-major host layouts so every DMA
reads/writes per-partition-contiguous blocks (the strided row layouts
were descriptor-inefficient and made iteration 1 DMA-bound).

Cost-model (TimelineSim) predicted NEFF time: ~490 us per core; PE busy
~471 us (96% occupancy), i.e. at the bf16 TensorE streaming floor, with
zero PE idle between steady-state iterations.
"""

import sys

for _p in ("/opt/trn_rl_repo",):
    if _p not in sys.path:
        sys.path.insert(0, _p)

import numpy as np
import ml_dtypes

import concourse.bass as bass
import concourse.mybir as mybir
import concourse.tile as tile
from concourse import bacc
from concourse.bass_utils import run_bass_kernel_spmd
from concourse.masks import make_identity

B, M, N, K = 2048, 1024, 4096, 10
EPS = 0.01
NCORES = 8
BL = B // NCORES            # 256 batch rows per core
P = 128
MT = M // P                 # 8 m-tiles
NT = N // P                 # 32 n-tiles
MM2_DELAY = 4               # emission lag of matmul2 behind matmul1 (in nt units)
MM2_DELAY_K1 = 2            # same, for the DMA-paced first iteration
BETAY_ENGINE = "gpsimd"     # engine for the per-iter beta*y prep
SPLIT_W0 = False            # split first W strip into two DMAs
THREE_DMA_QUEUES = False    # AT on the scalar engine queue
LAST_ITER_ACT_CHAIN = True  # ScalarE-based |Z|(|Z|+eps) in the final iteration
DMA_PAIR_STRIPS = False     # W/AT input DMAs as 2-strip transfers
EW_BUFS = 4
PS1_BUFS = 4
PADJ_ENGINE = "vector"      # which engine runs the p-adjust tensor_scalar

F32 = mybir.dt.float32
BF16 = mybir.dt.bfloat16
ALU = mybir.AluOpType
ACT = mybir.ActivationFunctionType


# ---- custom fused DVE ops ----
# ABS_SHRINK_P: p = max(|Z| * (|Z| + C0), C1)       (C0=EPS, C1=floor)
# RELU_AFF_MUL: out = relu(Src0*C0 + C1) * Src1     (C0=-mu, C1=1.0, Src1=Z)
from concourse import dve_ops as _dvo
from concourse.dve_spec import Spec as _Spec, Src0 as _S0, Src1 as _S1, \
    C0 as _C0, C1 as _C1, Zero as _Z0, relu as _relu, maxx as _maxx

def _register(name, spec):
    """Reserve an opcode row, discover the uops sha, register the op."""
    import re as _re
    if name in _dvo._SUB_OPCODE_FOR_NAME:
        return next(op for op in _dvo.OPS if op.name == name)
    row = _dvo._CUSTOM_DVE_ROW_BASE + len(_dvo.OPS)
    assert row < 0x20
    _dvo._SUB_OPCODE_FOR_NAME[name] = row
    shas = {}
    for ver in ("v3",):
        try:
            _dvo.DveOp(name, spec, subdim=False, uops_sha={}).compile(ver)
        except ValueError as e:
            m = _re.search(r"%s: ([0-9a-f]+)" % ver, str(e))
            if not m:
                raise
            shas[ver] = m.group(1)
    op = _dvo.DveOp(name, spec, subdim=False, uops_sha=shas)
    _dvo.OPS.append(op)
    _dvo.CUSTOM_DVE_SPECS[name] = spec
    return op

_absu = _maxx(_S0, _Z0 - _S0)
ABS_SHRINK_P = _register("ABS_SHRINK_P_ATASI", _Spec(
    body=_maxx(_absu * (_absu + _C0), _C1),
    reference=lambda in0, in1, s0, s1, imm2: np.maximum(
        np.abs(in0.astype(np.float32)) * (np.abs(in0.astype(np.float32)) + s0), s1),
))
RELU_AFF_MUL = _register("RELU_AFF_MUL_ATASI", _Spec(
    body=_relu(_S0 * _C0 + _C1) * _S1,
    reference=lambda in0, in1, s0, s1, imm2: np.maximum(
        in0.astype(np.float32) * s0 + s1, 0.0) * in1,
))

_cached_nc = None


def build(n_iters=K - 1, debug_dt=False, ncores=NCORES, n_reps=1):
    """n_reps > 1 wraps the whole per-call computation in a hardware For_i
    loop that re-runs it back-to-back (reloading the D state from dt0 between
    repetitions so every repetition is bit-identical to the n_reps=1 kernel).
    Used only for timing: executing the kernel R times inside one NEFF
    amortizes the multi-ms axon dispatch overhead, so the per-execution
    wall-clock slope measures the actual hardware execution time."""
    nc = bacc.Bacc("TRN2", target_bir_lowering=False, debug=False, num_devices=ncores)

    w_d = nc.dram_tensor("Wneg", [NT, P, MT, P], BF16, kind="ExternalInput")
    dt0_d = nc.dram_tensor("dt0", [P, MT, BL], BF16, kind="ExternalInput")
    at_d = nc.dram_tensor("AT", [N, M], BF16, kind="ExternalInput")
    yneg_d = nc.dram_tensor("ynegT", [P, MT, BL], BF16, kind="ExternalInput")
    negmu_d = nc.dram_tensor("negmu", [P, K], F32, kind="ExternalInput")
    beta_d = nc.dram_tensor("beta", [P, K], F32, kind="ExternalInput")
    out_d = nc.dram_tensor("out", [P, NT, BL], F32, kind="ExternalOutput")
    dt_d = (nc.dram_tensor("dt_out", [M, BL], BF16, kind="ExternalOutput")
            if debug_dt else None)

    with tile.TileContext(nc) as tc:
        with (
            tc.tile_pool(name="const", bufs=1) as cpool,
            tc.tile_pool(name="ew", bufs=EW_BUFS) as ew,
            tc.tile_pool(name="ps1", bufs=PS1_BUFS, space="PSUM") as ps1,
            tc.tile_pool(name="ps2", bufs=1, space="PSUM") as ps2,
        ):
            wsb = cpool.tile([P, NT, MT, P], BF16, tag="wsb")
            atsb = cpool.tile([P, NT, M], BF16, tag="atsb")
            ynegsb = cpool.tile([P, MT, BL], BF16, tag="ynegsb")
            negmu = cpool.tile([P, K], F32, tag="negmu")
            betasb = cpool.tile([P, K], F32, tag="betasb")
            gb = cpool.tile([P, NT, BL], BF16, tag="gb")        # gamma state (bf16)
            dt = cpool.tile([P, MT, BL], BF16, tag="dt")        # beta_k * D^T
            betay = cpool.tile([P, MT, BL], BF16, tag="betay")  # beta_{k+1} * (-y^T)
            ident = cpool.tile([P, P], BF16, tag="ident")

            # first matmul needs dt0 + W strip 0; lead with those on
            # separate queues, defer everything not needed until later.
            nc.sync.dma_start(wsb[:, 0], w_d.ap()[0])
            nc.gpsimd.dma_start(dt[:], dt0_d.ap())
            nc.scalar.dma_start(negmu[:], negmu_d[:])
            nc.scalar.dma_start(betasb[:], beta_d[:])

            halfeps = cpool.tile([P, 1], F32, tag="halfeps")
            nc.vector.memset(halfeps[:], EPS / 2)
            make_identity(nc, ident[:])
            dma_engines = [nc.sync, nc.gpsimd]
            if DMA_PAIR_STRIPS:
                # fewer, larger transfers: W strips in pairs (1 MB), AT in
                # pairs (512 KB), alternating queues, arrival-ordered.
                nc.gpsimd.dma_start(
                    wsb[:, 1], w_d.ap()[1].rearrange("(o p) c -> p o c", p=P)
                )
                nc.sync.dma_start(atsb[:, 0:2], at_d.ap()[0:2 * P, :]
                                  .rearrange("(o p) m -> p o m", p=P))
                for i in range(1, NT // 2):
                    nt = 2 * i
                    dma_engines[i % 2].dma_start(
                        wsb[:, nt:nt + 2],
                        w_d.ap()[nt:nt + 2].rearrange("o (oo p) c -> p o oo c", p=P),
                    )
                    dma_engines[(i + 1) % 2].dma_start(
                        atsb[:, nt:nt + 2], at_d.ap()[nt * P:(nt + 2) * P, :]
                        .rearrange("(o p) m -> p o m", p=P),
                    )
            else:
                # each W strip split across both queues halves its arrival
                # latency; AT strips alternate queues one step behind.
                for nt in range(1, NT):
                    nc.sync.dma_start(wsb[:, nt, 0:4], w_d.ap()[nt, :, 0:4])
                    nc.gpsimd.dma_start(wsb[:, nt, 4:8], w_d.ap()[nt, :, 4:8])
                    dma_engines[(nt + 1) % 2].dma_start(
                        atsb[:, nt - 1], at_d.ap()[(nt - 1) * P:nt * P, :]
                    )
                nc.sync.dma_start(atsb[:, NT - 1], at_d.ap()[(NT - 1) * P:NT * P, :])
            nc.scalar.dma_start(ynegsb[:], yneg_d.ap())

            out_v = out_d.ap()

            def emit_body(reload_dt):
              def emit_mm2(k, nt):
                """matmul2: accumulate A@gammaT for this nt into all 8 D slices.

                Two m-slices share one PSUM bank; start=True clears the WHOLE
                bank, so only the first (even) slice of each pair may set it.
                The odd slice's first write lands on cleared has_written bits
                and overwrites anyway."""
                for mt in range(MT):
                    dacc = dpsum[mt // 2][:, (mt % 2) * BL:(mt % 2) * BL + BL]
                    nc.tensor.matmul(
                        dacc,
                        atsb[:, nt, mt * P:(mt + 1) * P],
                        gb[:, nt],
                        start=(nt == 0 and mt % 2 == 0),
                        stop=(nt == NT - 1),
                        skip_group_check=True,
                    )

              for k in range(1, 1 + n_iters):
                last = (k == n_iters)
                if last and debug_dt:
                    last = False  # still run the D update so we can dump it
                if not last:
                    # prep beta_{k+1} * (-y)^T for the D epilogue
                    for mt in range(MT):
                        getattr(nc, BETAY_ENGINE).tensor_scalar_mul(
                            betay[:, mt], ynegsb[:, mt], betasb[:, k + 1:k + 2]
                        )
                    # paired D accumulators: 4 banks x [128, 512] hold 8 m-slices
                    dpsum = [
                        ps2.tile([P, 2 * BL], F32, tag=f"dps{j}", name=f"dps{j}_{k}")
                        for j in range(4)
                    ]

                delay = MM2_DELAY_K1 if k == 1 else MM2_DELAY
                for nt in range(NT):
                    # U^T accumulates in PSUM: sum_m (-W)^T (beta D)^T; the
                    # gamma term is added on PE (identity matmul) only when
                    # the VectorE is the pacer (last iteration); otherwise
                    # VectorE does the subtract, freeing PE cycles.
                    use_ident = (k > 1) and last
                    z = ps1.tile([P, BL], F32, tag="z")
                    if use_ident:
                        nc.tensor.matmul(
                            z, ident[:], gb[:, nt],
                            start=True, stop=False, skip_group_check=True,
                        )
                    for mt in range(MT):
                        nc.tensor.matmul(
                            z,
                            wsb[:, nt, mt],
                            dt[:, mt],
                            start=(mt == 0 and not use_ident),
                            stop=(mt == MT - 1),
                            skip_group_check=True,
                        )
                    if (not use_ident) and k > 1:
                        zs = ew.tile([P, BL], F32, tag="zs")
                        nc.vector.tensor_tensor(zs, gb[:, nt], z, ALU.add)
                        z = zs
                    pp = ew.tile([P, BL], F32, tag="pp")
                    if last and LAST_ITER_ACT_CHAIN:
                        # no mm2 in the last iteration, so DVE (not PE) paces
                        # it; compute p on ScalarE to rebalance.
                        az = ew.tile([P, BL], F32, tag="az")
                        nc.scalar.activation(az, z, ACT.Abs)
                        nc.scalar.activation(pp, az, ACT.Square,
                                             bias=halfeps[:, 0:1])
                        nc.vector.tensor_scalar(
                            pp, pp, -EPS * EPS / 4, 1e-30, ALU.add, ALU.max)
                    else:
                        nc.vector._custom_dve(
                            ABS_SHRINK_P, out=pp, in0=z, s0=EPS, s1=1e-30)
                    r = ew.tile([P, BL], F32, tag="r")
                    nc.vector.reciprocal_approx_fast(r, pp)
                    # gamma = relu(1 - mu/p) * Z
                    if last:
                        gstage = ew.tile([P, BL], F32, tag="gstage")
                        nc.vector._custom_dve(
                            RELU_AFF_MUL, out=gstage, in0=r, in1=z,
                            s0=negmu[:, k:k + 1], s1=1.0)
                        dma_engines[nt % 2].dma_start(out_v[:, nt], gstage[:])
                    else:
                        nc.vector._custom_dve(
                            RELU_AFF_MUL, out=gb[:, nt], in0=r, in1=z,
                            s0=negmu[:, k:k + 1], s1=1.0)
                        if nt >= delay:
                            emit_mm2(k, nt - delay)

                if not last:
                    for nt in range(NT - delay, NT):
                        emit_mm2(k, nt)
                    # D epilogue: DT = beta_{k+1}*P + beta_{k+1}*(-y),
                    # one op per PSUM bank (two m-slices at FD=512)
                    for j in range(0, 4):
                        nc.vector.scalar_tensor_tensor(
                            dt[:, 2 * j:2 * j + 2], dpsum[j][:],
                            betasb[:, k + 1:k + 2], betay[:, 2 * j:2 * j + 2],
                            ALU.mult, ALU.add,
                        )
              # re-arm D state for the next repetition; emitted after the
              # final iteration's matmuls have consumed dt, so this DMA
              # overlaps the trailing elementwise/output work.
              if reload_dt:
                nc.scalar.dma_start(dt[:], dt0_d.ap())

            if n_reps == 1:
                emit_body(False)
            else:
                with tc.For_i(0, n_reps, 1):
                    emit_body(True)

            if debug_dt:
                for nt in range(NT):
                    gstage2 = ew.tile([P, BL], F32, tag="gstage2", name=f"gs2_{nt}")
                    nc.vector.tensor_copy(out=gstage2[:], in_=gb[:, nt])
                    nc.sync.dma_start(out_v[:, nt], gstage2[:])
                nc.sync.dma_start(
                    dt_d.ap().rearrange("(o p) b -> p o b", p=P), dt[:]
                )

    nc.compile()
    return nc


def _kernel_impl(y, A, W, mu, beta):
    global _cached_nc
    # np.asarray on jax device arrays triggers a device->host transfer, so
    # this conversion must also be inside the retry guard below.
    y = np.asarray(y, np.float32)
    A = np.asarray(A, np.float32)
    W = np.asarray(W, np.float32)
    mu = np.asarray(mu, np.float32)
    beta = np.asarray(beta, np.float32)

    if _cached_nc is None:
        _cached_nc = build()
    nc = _cached_nc

    in_maps = make_in_maps(y, A, W, mu, beta)
    res = run_bass_kernel_spmd(nc, in_maps, core_ids=list(range(NCORES)))
    # out per core: [P, NT, BL] with n = nt*128 + p -> gather to [B, N]
    return np.concatenate(
        [r["out"].transpose(2, 1, 0).reshape(BL, N) for r in res.results], axis=0)


def kernel(y, A, W, mu, beta):
    try:
        return _kernel_impl(y, A, W, mu, beta)
    except Exception:
        # The axon-tunneled devices occasionally come up unrecoverable right
        # after a previous process's teardown; reset the backend and retry
        # everything, including any device->host input transfers.
        import time as _time
        _time.sleep(15)
        try:
            import jax as _jax
            _jax.clear_caches()
            _jax.extend.backend.clear_backends()
        except Exception:
            pass
        return _kernel_impl(y, A, W, mu, beta)


def make_in_maps(y, A, W, mu, beta):
    # Wneg[nt, p, o, c] = -W[o*128+p, nt*128+c]: every DMA reads
    # per-partition-contiguous blocks.
    wb = np.ascontiguousarray(
        (-W).astype(ml_dtypes.bfloat16)
        .reshape(MT, P, NT, P).transpose(2, 1, 0, 3))
    atb = np.ascontiguousarray(A.T).astype(ml_dtypes.bfloat16)
    ynegt = np.ascontiguousarray(-y.T).astype(ml_dtypes.bfloat16)  # [M, B]
    negmu_b = np.ascontiguousarray(np.broadcast_to(-mu, (P, K))).astype(np.float32)
    beta_b = np.ascontiguousarray(np.broadcast_to(beta, (P, K))).astype(np.float32)
    in_maps = []
    for c in range(NCORES):
        yc = np.ascontiguousarray(
            ynegt[:, c * BL:(c + 1) * BL].reshape(MT, P, BL).transpose(1, 0, 2))
        in_maps.append({
            "Wneg": wb,
            "AT": atb,
            "ynegT": yc,
            "dt0": (yc.astype(np.float32) * beta[1]).astype(ml_dtypes.bfloat16),
            "negmu": negmu_b,
            "beta": beta_b,
        })
    return in_maps


if __name__ == "__main__":
    rng = np.random.default_rng(0)
    y = rng.standard_normal((B, M)).astype(np.float32)
    A = (rng.standard_normal((M, N)) / np.sqrt(M)).astype(np.float32)
    W = (rng.standard_normal((M, N)) / np.sqrt(M)).astype(np.float32)
    mu = rng.random(K).astype(np.float32)
    beta = rng.random(K).astype(np.float32)
    g = kernel(y=y, A=A, W=W, mu=mu, beta=beta)
    print("out", g.shape, g.dtype, np.abs(g).max())



# revision 21
# speedup vs baseline: 1.0365x; 1.0365x over previous
"""LISTA / AtasiNet Trainium2 kernel.

Math (reference): K=10 iterations of
    Z     = gamma - D @ (beta_k * W)          # [B,N]
    theta = mu_k / (|Z| + EPS)
    gamma = sign(Z) * max(|Z| - theta, 0)
    D     = gamma @ A.T - y                   # [B,M]

Distribution: pure data-parallel over batch B=2048 across 8 NeuronCores
(B_local=256 per core); A, W, mu, beta replicated. No collectives.

Per-core layout is fully transposed (batch on the free axis):
    gammaT [N=4096, B=256]   DT [M=1024, B=256]
so both matmuls take naturally-laid-out weights:
    Z^T = gammaT + (-W)^T @ (beta_k D)^T : lhsT = -W tile  [m128, n128]
    D^T = A @ gammaT                     : lhsT = A.T tile [n128, m128]
A.T and -W are precomputed on host; matmul inputs are bf16 with f32 PSUM
accumulation. The gamma subtraction runs on VectorE (which has slack) for
iterations 2-8; in the final iteration - where VectorE would be the pacer
because there is no D update - it is instead folded into the matmul PSUM
accumulation group as an identity matmul of the bf16 gamma state.

Iteration k=0 is folded out analytically: for any mu_0 >= 0 the first
iteration yields gamma=0, D=-y. Device loop runs k=1..9; the last
iteration skips the (unused) D update and writes gamma out in f32.

Elementwise threshold uses the multiplicative form
    gamma = Z * relu(1 - mu_k / (|Z| (|Z|+EPS)))
as three VectorE ops per tile: a custom fused DVE op for
p = max(|Z|(|Z|+EPS), 1e-30), reciprocal_approx_fast (~51 ULP), and a
custom fused relu-affine-multiply writing the bf16 gamma state. The final
iteration (no D update, so VectorE would pace it) moves the p computation
to ScalarE via Square(|Z| + EPS/2) - EPS^2/4.

All DRAM inputs/outputs use partition-major host layouts so every DMA
reads/writes per-partition-contiguous blocks (the strided row layouts
were descriptor-inefficient and made iteration 1 DMA-bound).

Cost-model (TimelineSim) predicted NEFF time: ~490 us per core; PE busy
~471 us (96% occupancy), i.e. at the bf16 TensorE streaming floor, with
zero PE idle between steady-state iterations.
"""

import sys

for _p in ("/opt/trn_rl_repo",):
    if _p not in sys.path:
        sys.path.insert(0, _p)

import numpy as np
import ml_dtypes

import concourse.bass as bass
import concourse.mybir as mybir
import concourse.tile as tile
from concourse import bacc
from concourse.bass_utils import run_bass_kernel_spmd
from concourse.masks import make_identity

B, M, N, K = 2048, 1024, 4096, 10
EPS = 0.01
NCORES = 8
BL = B // NCORES            # 256 batch rows per core
P = 128
MT = M // P                 # 8 m-tiles
NT = N // P                 # 32 n-tiles
MM2_DELAY = 4               # emission lag of matmul2 behind matmul1 (in nt units)
MM2_DELAY_K1 = 2            # same, for the DMA-paced first iteration
BETAY_ENGINE = "scalar"     # engine for the per-iter beta*y prep (ACT idles in non-last iters; GPSIMD Q7 is slow and gated the D epilogue)
SPLIT_W0 = False            # split first W strip into two DMAs
THREE_DMA_QUEUES = False    # AT on the scalar engine queue
LAST_ITER_ACT_CHAIN = True  # ScalarE-based |Z|(|Z|+eps) in the final iteration
DMA_PAIR_STRIPS = False     # W/AT input DMAs as 2-strip transfers
EW_BUFS = 4
PS1_BUFS = 4
PADJ_ENGINE = "vector"      # which engine runs the p-adjust tensor_scalar

F32 = mybir.dt.float32
BF16 = mybir.dt.bfloat16
ALU = mybir.AluOpType
ACT = mybir.ActivationFunctionType


# ---- custom fused DVE ops ----
# ABS_SHRINK_P: p = max(|Z| * (|Z| + C0), C1)       (C0=EPS, C1=floor)
# RELU_AFF_MUL: out = relu(Src0*C0 + C1) * Src1     (C0=-mu, C1=1.0, Src1=Z)
from concourse import dve_ops as _dvo
from concourse.dve_spec import Spec as _Spec, Src0 as _S0, Src1 as _S1, \
    C0 as _C0, C1 as _C1, Zero as _Z0, relu as _relu, maxx as _maxx

def _register(name, spec):
    """Reserve an opcode row, discover the uops sha, register the op."""
    import re as _re
    if name in _dvo._SUB_OPCODE_FOR_NAME:
        return next(op for op in _dvo.OPS if op.name == name)
    row = _dvo._CUSTOM_DVE_ROW_BASE + len(_dvo.OPS)
    assert row < 0x20
    _dvo._SUB_OPCODE_FOR_NAME[name] = row
    shas = {}
    for ver in ("v3",):
        try:
            _dvo.DveOp(name, spec, subdim=False, uops_sha={}).compile(ver)
        except ValueError as e:
            m = _re.search(r"%s: ([0-9a-f]+)" % ver, str(e))
            if not m:
                raise
            shas[ver] = m.group(1)
    op = _dvo.DveOp(name, spec, subdim=False, uops_sha=shas)
    _dvo.OPS.append(op)
    _dvo.CUSTOM_DVE_SPECS[name] = spec
    return op

_absu = _maxx(_S0, _Z0 - _S0)
ABS_SHRINK_P = _register("ABS_SHRINK_P_ATASI", _Spec(
    body=_maxx(_absu * (_absu + _C0), _C1),
    reference=lambda in0, in1, s0, s1, imm2: np.maximum(
        np.abs(in0.astype(np.float32)) * (np.abs(in0.astype(np.float32)) + s0), s1),
))
RELU_AFF_MUL = _register("RELU_AFF_MUL_ATASI", _Spec(
    body=_relu(_S0 * _C0 + _C1) * _S1,
    reference=lambda in0, in1, s0, s1, imm2: np.maximum(
        in0.astype(np.float32) * s0 + s1, 0.0) * in1,
))

_cached_nc = None


def build(n_iters=K - 1, debug_dt=False, ncores=NCORES, n_reps=1,
          unroll_reps=False, loop_unroll=1, mm_split=1, probe_novec=False,
          probe_noout=False, probe_pe_only=False, probe_nss=False,
          pair_nt=False):
    """n_reps > 1 wraps the whole per-call computation in a hardware For_i
    loop that re-runs it back-to-back (reloading the D state from dt0 between
    repetitions so every repetition is bit-identical to the n_reps=1 kernel).
    Used only for timing: executing the kernel R times inside one NEFF
    amortizes the multi-ms axon dispatch overhead, so the per-execution
    wall-clock slope measures the actual hardware execution time."""
    nc = bacc.Bacc("TRN2", target_bir_lowering=False, debug=False, num_devices=ncores)

    w_d = nc.dram_tensor("Wneg", [NT, P, MT, P], BF16, kind="ExternalInput")
    dt0_d = nc.dram_tensor("dt0", [P, MT, BL], BF16, kind="ExternalInput")
    at_d = nc.dram_tensor("AT", [N, M], BF16, kind="ExternalInput")
    yneg_d = nc.dram_tensor("ynegT", [P, MT, BL], BF16, kind="ExternalInput")
    negmu_d = nc.dram_tensor("negmu", [P, K], F32, kind="ExternalInput")
    beta_d = nc.dram_tensor("beta", [P, K], F32, kind="ExternalInput")
    out_d = nc.dram_tensor("out", [P, NT, BL], F32, kind="ExternalOutput")
    dt_d = (nc.dram_tensor("dt_out", [M, BL], BF16, kind="ExternalOutput")
            if debug_dt else None)

    with tile.TileContext(nc) as tc:
        with (
            tc.tile_pool(name="const", bufs=1) as cpool,
            tc.tile_pool(name="ew", bufs=EW_BUFS) as ew,
            tc.tile_pool(name="ps1", bufs=PS1_BUFS, space="PSUM") as ps1,
            tc.tile_pool(name="ps2", bufs=1, space="PSUM") as ps2,
        ):
            wsb = cpool.tile([P, NT, MT, P], BF16, tag="wsb")
            atsb = cpool.tile([P, NT, M], BF16, tag="atsb")
            ynegsb = cpool.tile([P, MT, BL], BF16, tag="ynegsb")
            negmu = cpool.tile([P, K], F32, tag="negmu")
            betasb = cpool.tile([P, K], F32, tag="betasb")
            gb = cpool.tile([P, NT, BL], BF16, tag="gb")        # gamma state (bf16)
            dt0 = cpool.tile([P, MT, BL], BF16, tag="dt")       # beta_k * D^T
            # second D buffer: repetitions alternate dt0/dt1 so the dt0-state
            # reload for rep r+1 streams in with no WAR hazard against rep
            # r's matmul reads (its target was last read two reps ago).
            dt1 = cpool.tile([P, MT, BL], BF16, tag="dtb")
            betay = cpool.tile([P, MT, BL], BF16, tag="betay")  # beta_{k+1} * (-y^T)
            ident = cpool.tile([P, P], BF16, tag="ident")
            dt = dt0

            # first matmul needs dt0 + W strip 0; lead with those on
            # separate queues, defer everything not needed until later.
            nc.sync.dma_start(wsb[:, 0], w_d.ap()[0])
            nc.gpsimd.dma_start(dt[:], dt0_d.ap())
            nc.scalar.dma_start(negmu[:], negmu_d[:])
            nc.scalar.dma_start(betasb[:], beta_d[:])

            halfeps = cpool.tile([P, 1], F32, tag="halfeps")
            nc.vector.memset(halfeps[:], EPS / 2)
            if probe_pe_only:
                nc.vector.memset(gb[:], 0.001)
            make_identity(nc, ident[:])
            dma_engines = [nc.sync, nc.gpsimd]
            if DMA_PAIR_STRIPS:
                # fewer, larger transfers: W strips in pairs (1 MB), AT in
                # pairs (512 KB), alternating queues, arrival-ordered.
                nc.gpsimd.dma_start(
                    wsb[:, 1], w_d.ap()[1].rearrange("(o p) c -> p o c", p=P)
                )
                nc.sync.dma_start(atsb[:, 0:2], at_d.ap()[0:2 * P, :]
                                  .rearrange("(o p) m -> p o m", p=P))
                for i in range(1, NT // 2):
                    nt = 2 * i
                    dma_engines[i % 2].dma_start(
                        wsb[:, nt:nt + 2],
                        w_d.ap()[nt:nt + 2].rearrange("o (oo p) c -> p o oo c", p=P),
                    )
                    dma_engines[(i + 1) % 2].dma_start(
                        atsb[:, nt:nt + 2], at_d.ap()[nt * P:(nt + 2) * P, :]
                        .rearrange("(o p) m -> p o m", p=P),
                    )
            else:
                # each W strip split across both queues halves its arrival
                # latency; AT strips alternate queues one step behind.
                for nt in range(1, NT):
                    nc.sync.dma_start(wsb[:, nt, 0:4], w_d.ap()[nt, :, 0:4])
                    nc.gpsimd.dma_start(wsb[:, nt, 4:8], w_d.ap()[nt, :, 4:8])
                    dma_engines[(nt + 1) % 2].dma_start(
                        atsb[:, nt - 1], at_d.ap()[(nt - 1) * P:nt * P, :]
                    )
                nc.sync.dma_start(atsb[:, NT - 1], at_d.ap()[(NT - 1) * P:NT * P, :])
            nc.scalar.dma_start(ynegsb[:], yneg_d.ap())

            out_v = out_d.ap()

            def emit_body(reload_dt, dt=dt0, dtn=dt0):
              # re-arm the NEXT repetition's D buffer as early as possible:
              # its last reader finished two repetitions ago, so the DMA
              # overlaps this repetition's compute instead of stalling the
              # next one.
              if reload_dt:
                nc.scalar.dma_start(dtn[:], dt0_d.ap())

              def emit_mm2(k, nt):
                """matmul2: accumulate A@gammaT for this nt into all 8 D slices.

                Two m-slices share one PSUM bank; start=True clears the WHOLE
                bank, so only the first (even) slice of each pair may set it.
                The odd slice's first write lands on cleared has_written bits
                and overwrites anyway."""
                for mt in range(MT):
                    dacc = dpsum[mt // 2][:, (mt % 2) * BL:(mt % 2) * BL + BL]
                    fs = BL // mm_split
                    for s in range(mm_split):
                        nc.tensor.matmul(
                            dacc[:, s * fs:(s + 1) * fs],
                            atsb[:, nt, mt * P:(mt + 1) * P],
                            gb[:, nt, s * fs:(s + 1) * fs],
                            start=(nt == 0 and mt % 2 == 0 and s == 0),
                            stop=(nt == NT - 1),
                            skip_group_check=True,
                        )

              for k in range(1, 1 + n_iters):
                last = (k == n_iters)
                if last and debug_dt:
                    last = False  # still run the D update so we can dump it
                if not last:
                    # prep beta_{k+1} * (-y)^T for the D epilogue
                    if not probe_pe_only:
                        for mt in range(MT):
                            if BETAY_ENGINE == "scalar":
                                # ACT idles in non-last iterations; GPSIMD's
                                # Q7 software mul was slow enough to gate the
                                # D epilogue and stall the next iteration.
                                nc.scalar.activation(
                                    betay[:, mt], ynegsb[:, mt], ACT.Copy,
                                    scale=betasb[:, k + 1:k + 2],
                                )
                            else:
                                getattr(nc, BETAY_ENGINE).tensor_scalar_mul(
                                    betay[:, mt], ynegsb[:, mt],
                                    betasb[:, k + 1:k + 2],
                                )
                    # paired D accumulators: 4 banks x [128, 512] hold 8 m-slices
                    dpsum = [
                        ps2.tile([P, 2 * BL], F32, tag=f"dps{j}", name=f"dps{j}_{k}")
                        for j in range(4)
                    ]

                delay = MM2_DELAY_K1 if k == 1 else MM2_DELAY
                if pair_nt:
                    # nt-PAIR mode: one full PSUM bank [P, 2*BL] accumulates
                    # two adjacent n-tiles per group; the elementwise chain
                    # runs on [P, 512] tiles (half the DVE instructions and
                    # semaphores), and the final-iteration output DMAs move
                    # 2 KB per partition instead of 1 KB.
                    assert mm_split == 1
                    use_ident = (k > 1) and last
                    pdelay = max(1, delay // 2)
                    for pt in range(NT // 2):
                        z = ps1.tile([P, 2 * BL], F32, tag="z")
                        for h in range(2):
                            nt = 2 * pt + h
                            zh = z[:, h * BL:(h + 1) * BL]
                            if use_ident:
                                nc.tensor.matmul(
                                    zh, ident[:], gb[:, nt],
                                    start=(h == 0), stop=False,
                                    skip_group_check=True,
                                )
                            for mt in range(MT):
                                nc.tensor.matmul(
                                    zh, wsb[:, nt, mt], dt[:, mt],
                                    start=(h == 0 and mt == 0 and not use_ident),
                                    stop=(h == 1 and mt == MT - 1),
                                    skip_group_check=True,
                                )
                        gpair = gb[:, 2 * pt:2 * pt + 2]
                        zin = z
                        if (not use_ident) and k > 1:
                            zs = ew.tile([P, 2 * BL], F32, tag="zs")
                            nc.vector.tensor_tensor(zs, gpair, z, ALU.add)
                            zin = zs
                        pp = ew.tile([P, 2 * BL], F32, tag="pp")
                        if last and LAST_ITER_ACT_CHAIN:
                            az = ew.tile([P, 2 * BL], F32, tag="az")
                            nc.scalar.activation(az, zin, ACT.Abs)
                            nc.scalar.activation(pp, az, ACT.Square,
                                                 bias=halfeps[:, 0:1])
                            nc.vector.tensor_scalar(
                                pp, pp, -EPS * EPS / 4, 1e-30, ALU.add, ALU.max)
                        else:
                            nc.vector._custom_dve(
                                ABS_SHRINK_P, out=pp, in0=zin, s0=EPS, s1=1e-30)
                        r = ew.tile([P, 2 * BL], F32, tag="r")
                        nc.vector.reciprocal_approx_fast(r, pp)
                        if last:
                            gstage = ew.tile([P, 2 * BL], F32, tag="gstage")
                            nc.vector._custom_dve(
                                RELU_AFF_MUL, out=gstage, in0=r, in1=zin,
                                s0=negmu[:, k:k + 1], s1=1.0)
                            if not probe_noout:
                                dma_engines[pt % 2].dma_start(
                                    out_v[:, 2 * pt:2 * pt + 2], gstage[:])
                        else:
                            nc.vector._custom_dve(
                                RELU_AFF_MUL, out=gpair, in0=r, in1=zin,
                                s0=negmu[:, k:k + 1], s1=1.0)
                            if pt >= pdelay:
                                emit_mm2(k, 2 * (pt - pdelay))
                                emit_mm2(k, 2 * (pt - pdelay) + 1)
                    if not last:
                        for pt in range(NT // 2 - pdelay, NT // 2):
                            emit_mm2(k, 2 * pt)
                            emit_mm2(k, 2 * pt + 1)
                        for j in range(0, 4):
                            nc.vector.scalar_tensor_tensor(
                                dt[:, 2 * j:2 * j + 2], dpsum[j][:],
                                betasb[:, k + 1:k + 2], betay[:, 2 * j:2 * j + 2],
                                ALU.mult, ALU.add,
                            )
                    continue
                for nt in range(NT):
                    # U^T accumulates in PSUM: sum_m (-W)^T (beta D)^T; the
                    # gamma term is added on PE (identity matmul) only when
                    # the VectorE is the pacer (last iteration); otherwise
                    # VectorE does the subtract, freeing PE cycles.
                    use_ident = (k > 1) and last
                    z = ps1.tile([P, BL], F32, tag="z")
                    fs = BL // mm_split
                    for s in range(mm_split):
                        if use_ident:
                            nc.tensor.matmul(
                                z[:, s * fs:(s + 1) * fs], ident[:],
                                gb[:, nt, s * fs:(s + 1) * fs],
                                start=(s == 0), stop=False,
                                skip_group_check=True,
                            )
                    for mt in range(MT):
                        for s in range(mm_split):
                            if probe_nss:
                                # timing probe: one accumulation group per
                                # PSUM bank per iteration (numerics wrong)
                                st = (nt < PS1_BUFS and mt == 0 and s == 0
                                      and not use_ident)
                                sp = (nt >= NT - PS1_BUFS and mt == MT - 1)
                            else:
                                st = (mt == 0 and s == 0 and not use_ident)
                                sp = (mt == MT - 1)
                            nc.tensor.matmul(
                                z[:, s * fs:(s + 1) * fs],
                                wsb[:, nt, mt],
                                dt[:, mt, s * fs:(s + 1) * fs],
                                start=st,
                                stop=sp,
                                skip_group_check=True,
                            )
                    if probe_pe_only:
                        if not last:
                            if nt >= delay:
                                emit_mm2(k, nt - delay)
                        continue
                    if probe_novec:
                        # timing probe: single DVE copy instead of the
                        # 4-op elementwise chain (numerics intentionally wrong)
                        if last:
                            gstage = ew.tile([P, BL], F32, tag="gstage")
                            nc.vector.tensor_copy(out=gstage[:], in_=z)
                            if not probe_noout:
                                dma_engines[nt % 2].dma_start(out_v[:, nt], gstage[:])
                        else:
                            nc.vector.tensor_copy(out=gb[:, nt], in_=z)
                            if nt >= delay:
                                emit_mm2(k, nt - delay)
                        continue
                    if (not use_ident) and k > 1:
                        zs = ew.tile([P, BL], F32, tag="zs")
                        nc.vector.tensor_tensor(zs, gb[:, nt], z, ALU.add)
                        z = zs
                    pp = ew.tile([P, BL], F32, tag="pp")
                    if last and LAST_ITER_ACT_CHAIN:
                        # no mm2 in the last iteration, so DVE (not PE) paces
                        # it; compute p on ScalarE to rebalance.
                        az = ew.tile([P, BL], F32, tag="az")
                        nc.scalar.activation(az, z, ACT.Abs)
                        nc.scalar.activation(pp, az, ACT.Square,
                                             bias=halfeps[:, 0:1])
                        nc.vector.tensor_scalar(
                            pp, pp, -EPS * EPS / 4, 1e-30, ALU.add, ALU.max)
                    else:
                        nc.vector._custom_dve(
                            ABS_SHRINK_P, out=pp, in0=z, s0=EPS, s1=1e-30)
                    r = ew.tile([P, BL], F32, tag="r")
                    nc.vector.reciprocal_approx_fast(r, pp)
                    # gamma = relu(1 - mu/p) * Z
                    if last:
                        gstage = ew.tile([P, BL], F32, tag="gstage")
                        nc.vector._custom_dve(
                            RELU_AFF_MUL, out=gstage, in0=r, in1=z,
                            s0=negmu[:, k:k + 1], s1=1.0)
                        if not probe_noout:
                            dma_engines[nt % 2].dma_start(out_v[:, nt], gstage[:])
                    else:
                        nc.vector._custom_dve(
                            RELU_AFF_MUL, out=gb[:, nt], in0=r, in1=z,
                            s0=negmu[:, k:k + 1], s1=1.0)
                        if nt >= delay:
                            emit_mm2(k, nt - delay)

                if not last:
                    for nt in range(NT - delay, NT):
                        emit_mm2(k, nt)
                    # D epilogue: DT = beta_{k+1}*P + beta_{k+1}*(-y),
                    # one op per PSUM bank (two m-slices at FD=512)
                    for j in range(0, 4):
                        if probe_pe_only:
                            break
                        nc.vector.scalar_tensor_tensor(
                            dt[:, 2 * j:2 * j + 2], dpsum[j][:],
                            betasb[:, k + 1:k + 2], betay[:, 2 * j:2 * j + 2],
                            ALU.mult, ALU.add,
                        )
            dbuf = [dt0, dt1]
            if n_reps == 1:
                emit_body(False)
            elif unroll_reps:
                # fully unrolled repetitions (no hardware loop / no barrier):
                # lets TimelineSim measure the steady-state per-rep cost and
                # gives the Tile scheduler cross-repetition overlap.
                for r in range(n_reps):
                    emit_body(True, dbuf[r % 2], dbuf[(r + 1) % 2])
            else:
                # loop_unroll reps per For_i iteration: the all-engine
                # barrier + semaphore reset runs once per loop iteration, so
                # unrolling dilutes its cost and lets consecutive reps
                # overlap (next rep's matmuls start during the previous
                # rep's elementwise/output tail).
                assert n_reps % loop_unroll == 0
                if loop_unroll % 2:
                    # odd unroll cannot alternate dt buffers across the
                    # loop back-edge; fall back to single-buffer reload
                    with tc.For_i(0, n_reps // loop_unroll, 1):
                        for _ in range(loop_unroll):
                            emit_body(True)
                else:
                    with tc.For_i(0, n_reps // loop_unroll, 1):
                        for r in range(loop_unroll):
                            emit_body(True, dbuf[r % 2], dbuf[(r + 1) % 2])

            if debug_dt:
                for nt in range(NT):
                    gstage2 = ew.tile([P, BL], F32, tag="gstage2", name=f"gs2_{nt}")
                    nc.vector.tensor_copy(out=gstage2[:], in_=gb[:, nt])
                    nc.sync.dma_start(out_v[:, nt], gstage2[:])
                nc.sync.dma_start(
                    dt_d.ap().rearrange("(o p) b -> p o b", p=P), dt[:]
                )

    nc.compile()
    return nc


def _kernel_impl(y, A, W, mu, beta):
    global _cached_nc
    # np.asarray on jax device arrays triggers a device->host transfer, so
    # this conversion must also be inside the retry guard below.
    y = np.asarray(y, np.float32)
    A = np.asarray(A, np.float32)
    W = np.asarray(W, np.float32)
    mu = np.asarray(mu, np.float32)
    beta = np.asarray(beta, np.float32)

    if _cached_nc is None:
        _cached_nc = build()
    nc = _cached_nc

    in_maps = make_in_maps(y, A, W, mu, beta)
    res = run_bass_kernel_spmd(nc, in_maps, core_ids=list(range(NCORES)))
    # out per core: [P, NT, BL] with n = nt*128 + p -> gather to [B, N]
    return np.concatenate(
        [r["out"].transpose(2, 1, 0).reshape(BL, N) for r in res.results], axis=0)


def kernel(y, A, W, mu, beta):
    try:
        return _kernel_impl(y, A, W, mu, beta)
    except Exception:
        # The axon-tunneled devices occasionally come up unrecoverable right
        # after a previous process's teardown; reset the backend and retry
        # everything, including any device->host input transfers.
        import time as _time
        _time.sleep(15)
        try:
            import jax as _jax
            _jax.clear_caches()
            _jax.extend.backend.clear_backends()
        except Exception:
            pass
        return _kernel_impl(y, A, W, mu, beta)


def make_in_maps(y, A, W, mu, beta):
    # Wneg[nt, p, o, c] = -W[o*128+p, nt*128+c]: every DMA reads
    # per-partition-contiguous blocks.
    wb = np.ascontiguousarray(
        (-W).astype(ml_dtypes.bfloat16)
        .reshape(MT, P, NT, P).transpose(2, 1, 0, 3))
    atb = np.ascontiguousarray(A.T).astype(ml_dtypes.bfloat16)
    ynegt = np.ascontiguousarray(-y.T).astype(ml_dtypes.bfloat16)  # [M, B]
    negmu_b = np.ascontiguousarray(np.broadcast_to(-mu, (P, K))).astype(np.float32)
    beta_b = np.ascontiguousarray(np.broadcast_to(beta, (P, K))).astype(np.float32)
    in_maps = []
    for c in range(NCORES):
        yc = np.ascontiguousarray(
            ynegt[:, c * BL:(c + 1) * BL].reshape(MT, P, BL).transpose(1, 0, 2))
        in_maps.append({
            "Wneg": wb,
            "AT": atb,
            "ynegT": yc,
            "dt0": (yc.astype(np.float32) * beta[1]).astype(ml_dtypes.bfloat16),
            "negmu": negmu_b,
            "beta": beta_b,
        })
    return in_maps


if __name__ == "__main__":
    rng = np.random.default_rng(0)
    y = rng.standard_normal((B, M)).astype(np.float32)
    A = (rng.standard_normal((M, N)) / np.sqrt(M)).astype(np.float32)
    W = (rng.standard_normal((M, N)) / np.sqrt(M)).astype(np.float32)
    mu = rng.random(K).astype(np.float32)
    beta = rng.random(K).astype(np.float32)
    g = kernel(y=y, A=A, W=W, mu=mu, beta=beta)
    print("out", g.shape, g.dtype, np.abs(g).max())

